# revision 33
# baseline (speedup 1.0000x reference)
"""Trainium2 Bass kernel for nn_GATNet (GraphDTA-style GAT network).

Self-contained: host-side sharding/prep + Bass/Tile program + SPMD runner.
Sharding: 8 cores, core c owns nodes [750c, 750c+750) = graphs [16c, 16c+16);
edges partitioned by dst core and sorted by dst; GAT weights replicated;
layer-2 features exchanged via on-device AllGather (f16 table, f32 logits
bit-packed); MLP head data-parallel.

Key structure (per core):
  cell-MLP streams early; per branch: L1 edge-softmax aggregation in x-space
  (78-dim) -> fused [W1 matmul + ELU + W2 matmul] without materializing h1
  (ELU stored as h1+1 = min(exp(y), relu(y)+1); the -1 shift folds exactly
  through the layer-2 softmax aggregation into b2/bg/logit biases) ->
  AllGather of layer-2 node table (hidden behind the other branch's compute)
  -> L2 edge softmax aggregation (adst2 fetched via a PE transpose + one-hot
  matmul instead of per-edge DMA gathers) -> global max pool -> MLP head.
"""
import numpy as np

import concourse.bacc as bacc
import concourse.tile as tile
import concourse.mybir as mybir
from concourse.bass import IndirectOffsetOnAxis
from concourse.masks import make_identity

F32 = mybir.dt.float32
F16 = mybir.dt.float16
I32 = mybir.dt.int32
Alu = mybir.AluOpType
Act = mybir.ActivationFunctionType

N_CORES = 8
N_NODES = 6000
N_GRAPHS = 128
NV = 750
NBLK = 6
GPC = 16
H1, C1 = 10, 1024
D0 = 78
XTW = 128         # xtab row: 0:78 x | 78 one | 79:89 asrc | pad
T2W = 520         # table2 row: 0:512 h2p | 512 one | 513 pad | 514:516 asrc2(f32) | pad
NEG = 0.2         # PyG GATConv default negative_slope

HEAD_LAYERS = (("g", 512, 128), ("r1", 1024, 2048), ("r2", 2048, 512),
               ("r3", 512, 256), ("f1", 512, 1024), ("f2", 1024, 512),
               ("f3", 512, 128), ("o", 128, 2))


# ---------------------------------------------------------------- host prep
def prep_edges(ei):
    src = np.concatenate([ei[0], np.arange(N_NODES, dtype=ei.dtype)])
    dst = np.concatenate([ei[1], np.arange(N_NODES, dtype=ei.dtype)])
    order = np.argsort(dst, kind="stable")
    src, dst = src[order], dst[order]
    cores = []
    for c in range(N_CORES):
        m = (dst >= NV * c) & (dst < NV * (c + 1))
        s, d = src[m], dst[m] - NV * c
        blocks = []
        for b in range(NBLK):
            mb = (d >= 128 * b) & (d < min(128 * (b + 1), NV))
            blocks.append((s[mb], d[mb]))
        cores.append(blocks)
    n_ch = [max(1, int(np.ceil(max(len(cores[c][b][0]) for c in range(N_CORES))
                               / 128))) for b in range(NBLK)]
    nch_tot = sum(n_ch)
    src_idx = np.full((N_CORES, 128, nch_tot), 0, np.int32)
    dst_loc = np.full((N_CORES, 128, nch_tot), -1.0, np.float32)
    dst_glo = np.full((N_CORES, 128, nch_tot), 0, np.int64)
    for c in range(N_CORES):
        off = 0
        for b in range(NBLK):
            s, d = cores[c][b]
            npad = n_ch[b] * 128
            sp = np.full(npad, 0, np.int64)
            dp = np.full(npad, -1.0, np.float64)
            gp = np.full(npad, 0, np.int64)
            sp[:len(s)] = s
            dp[:len(d)] = d - 128 * b
            gp[:len(d)] = d + NV * c
            src_idx[c, :, off:off + n_ch[b]] = sp.reshape(n_ch[b], 128).T
            dst_loc[c, :, off:off + n_ch[b]] = dp.reshape(n_ch[b], 128).T
            dst_glo[c, :, off:off + n_ch[b]] = gp.reshape(n_ch[b], 128).T
            off += n_ch[b]
    return n_ch, src_idx, dst_loc, dst_glo


def host_prep(inputs):
    inp = {k: np.asarray(v) for k, v in inputs.items()}
    f16 = np.float16
    Hh = {}
    meta = {}
    W1 = inp["W1"].astype(np.float32)
    W13 = W1.reshape(D0, H1, C1)
    A_s = np.einsum("dhc,hc->dh", W13, inp["a_src1"].astype(np.float32))
    A_d = np.einsum("dhc,hc->dh", W13, inp["a_dst1"].astype(np.float32))
    for k in (1, 2):
        x = inp[f"x{k}"].astype(np.float32)
        asrc = x @ A_s
        adst = x @ A_d
        xtab = np.zeros((N_NODES + 16, XTW), f16)
        xtab[:N_NODES, 0:D0] = x
        xtab[:N_NODES, D0] = 1.0
        xtab[:N_NODES, 79:89] = asrc
        Hh[f"xtab{k}"] = xtab
        n_ch, src_idx, dst_loc, dst_glo = prep_edges(inp[f"edge_index{k}"])
        meta[f"n_ch{k}"] = n_ch
        Hh[f"srcidx{k}"] = src_idx
        Hh[f"dstloc{k}"] = dst_loc
        # local dst index (0..749) per edge chunk for the adst gather; pad -> 0
        dli = np.zeros((N_CORES, 128, sum(n_ch)), np.int32)
        for c in range(N_CORES):
            dli[c] = (dst_glo[c] - NV * c).astype(np.int64)
        dli[dst_loc < 0] = 0
        Hh[f"dstidx{k}"] = dli
        nch_tot = sum(n_ch)
        ad = np.zeros((N_CORES, 128, nch_tot * H1), f16)
        for c in range(N_CORES):
            ad[c] = adst[dst_glo[c].reshape(-1)].reshape(128, nch_tot * H1)
        Hh[f"adpe{k}"] = ad
        batch = inp[f"batch{k}"]
        bounds = np.searchsorted(batch, np.arange(N_GRAPHS + 1))
        for c in range(N_CORES):
            lb = bounds[GPC * c:GPC * (c + 1) + 1] - NV * c
            assert lb[0] == 0 and lb[-1] == NV, f"graphs not aligned: {lb}"
        meta[f"bounds{k}"] = (bounds[:GPC + 1]).tolist()
    Hh["W1f"] = W1.astype(f16)
    W2 = inp["W2"].astype(np.float32)
    wvs = W2 @ inp["a_src2"][0].astype(np.float32)
    wvd = W2 @ inp["a_dst2"][0].astype(np.float32)
    Hh["W2aug"] = W2.astype(f16)
    Hh["a2sb"] = np.broadcast_to(inp["a_src2"][0].astype(np.float32)[None, :],
                                 (128, 512)).copy()
    Hh["a2db"] = np.broadcast_to(inp["a_dst2"][0].astype(np.float32)[None, :],
                                 (128, 512)).copy()
    # h1 is stored as h1+1; corrections for the +1 shift:
    csd = float(wvs.sum() + wvd.sum())          # added to every L2 logit
    Hh["csdt"] = np.full((128, 1), -csd, np.float32)
    b1 = inp["b1"].astype(np.float32)
    Hh["b1t"] = b1.reshape(H1 * C1 // 128, 128).T.copy()           # [128, 80]
    Hh["b1p1t"] = Hh["b1t"] + 1.0
    b2c = inp["b2"].astype(np.float32) - W2.sum(axis=0)
    Hh["b2ct"] = b2c.reshape(4, 128).T.copy()                      # [128, 4]
    Hh["b2cp1t"] = Hh["b2ct"] + 1.0
    for nm, KD, MD in HEAD_LAYERS:
        wn = "W" + nm
        bn = "b" + nm
        KD0 = inp[wn].shape[0]
        W = np.zeros((KD, MD), f16)
        W[:KD0] = inp[wn].astype(f16)
        Hh[f"W{nm}"] = W
        bias = inp[bn].astype(np.float32)
        if nm == "g":  # o2 is stored +1; fold the correction into bg
            bias = bias - inp["Wg"].astype(np.float32).sum(axis=0)
        Hh[f"b{nm}"] = bias.reshape(MD, 1)
    cell = inp["cell"].astype(np.float32)
    cn = cell / np.maximum(np.linalg.norm(cell, axis=1, keepdims=True), 1e-12)
    cT = np.zeros((N_CORES, 1024, GPC), f16)
    for c in range(N_CORES):
        cT[c, :954] = cn[GPC * c:GPC * (c + 1)].T
    Hh["cellT"] = cT
    iota = np.broadcast_to(np.arange(128, dtype=f16)[None, :],
                           (128, 128)).copy()
    Hh["iotain"] = iota
    return Hh, meta


# ---------------------------------------------------------------- program
def build(Hh, meta, repeat=1, skip_cc=False, skip_gather=False):
    nc = bacc.Bacc("TRN2", target_bir_lowering=False, debug=False,
                   num_devices=N_CORES)

    def din(name, arr, dtype):
        return nc.dram_tensor(name, list(arr.shape), dtype,
                              kind="ExternalInput").ap()

    xtab = {k: din(f"xtab{k}", Hh[f"xtab{k}"], F16) for k in (1, 2)}
    W1f = din("W1f", Hh["W1f"], F16)
    W2aug = din("W2aug", Hh["W2aug"], F16)
    b1td = din("b1t", Hh["b1t"], F32)
    b1p1td = din("b1p1t", Hh["b1p1t"], F32)
    b2ctd = din("b2ct", Hh["b2ct"], F32)
    b2cp1td = din("b2cp1t", Hh["b2cp1t"], F32)
    csdtd = din("csdt", Hh["csdt"], F32)
    a2sbd = din("a2sb", Hh["a2sb"], F32)
    a2dbd = din("a2db", Hh["a2db"], F32)
    iotain = din("iotain", Hh["iotain"], F16)
    Wt = {nm: din(f"W{nm}", Hh[f"W{nm}"], F16) for nm, _, _ in HEAD_LAYERS}
    Bt = {nm: din(f"b{nm}", Hh[f"b{nm}"], F32) for nm, _, _ in HEAD_LAYERS}
    percore = {}
    for k in (1, 2):
        for nm, dtp in (("srcidx", I32), ("dstloc", F32), ("dstidx", I32),
                        ("adpe", F16)):
            arr = Hh[f"{nm}{k}"]
            percore[f"{nm}{k}"] = nc.dram_tensor(
                f"{nm}{k}", list(arr.shape[1:]), dtp, kind="ExternalInput").ap()
    cellT = nc.dram_tensor("cellT", list(Hh["cellT"].shape[1:]), F16,
                           kind="ExternalInput").ap()

    outT = nc.dram_tensor("outT", [2, GPC], F32, kind="ExternalOutput").ap()

    loc2 = {k: nc.dram_tensor(f"loc2_{k}", [NV, T2W], F16).ap() for k in (1, 2)}
    adstvd = {k: nc.dram_tensor(f"adstv{k}", [NBLK * 128, 1], F32).ap()
              for k in (1, 2)}
    shared2 = {k: nc.dram_tensor(f"shared2_{k}", [N_NODES, T2W], F16,
                                 addr_space="Shared").ap() for k in (1, 2)}

    n_ch = {k: meta[f"n_ch{k}"] for k in (1, 2)}
    nch_tot = {k: sum(n_ch[k]) for k in (1, 2)}
    NCHB = max(max(n_ch[1]), max(n_ch[2]))
    bounds = meta["bounds1"]
    assert meta["bounds2"] == bounds

    with tile.TileContext(nc) as tc:
        with (
            tc.tile_pool(name="const", bufs=1) as const,
            tc.tile_pool(name="headw", bufs=1) as headw,
            tc.tile_pool(name="aggp", bufs=1) as aggp,
            tc.tile_pool(name="o2p", bufs=1) as o2p,
            tc.tile_pool(name="stream", bufs=3) as stream,
            tc.tile_pool(name="xgpool", bufs=6) as xgpool,
            tc.tile_pool(name="hgpool", bufs=1) as hgpool,
            tc.tile_pool(name="sb", bufs=3) as sb,
        ):
            # ---------------- phase 0: constants + tables + weight preload
            sit, dlt, dit, adpe = {}, {}, {}, {}
            for k in (1, 2):
                NCH = nch_tot[k]
                sit[k] = const.tile([128, NCH], I32, tag=f"sit{k}", name=f"sit{k}")
                nc.sync.dma_start(out=sit[k][:], in_=percore[f"srcidx{k}"][:])
                dlt[k] = const.tile([128, NCH], F32, tag=f"dlt{k}", name=f"dlt{k}")
                nc.sync.dma_start(out=dlt[k][:], in_=percore[f"dstloc{k}"][:])
                dit[k] = const.tile([128, NCH], I32, tag=f"dit{k}", name=f"dit{k}")
                nc.sync.dma_start(out=dit[k][:], in_=percore[f"dstidx{k}"][:])
                adpe[k] = const.tile([128, NCH * H1], F16, tag=f"adpe{k}", name=f"adpe{k}")
                nc.sync.dma_start(out=adpe[k][:], in_=percore[f"adpe{k}"][:])
            iota = const.tile([128, 128], F16)
            nc.sync.dma_start(out=iota[:], in_=iotain[:])
            ident = const.tile([128, 128], F16)
            make_identity(nc, ident)
            b1t = const.tile([128, H1 * C1 // 128], F32)
            nc.sync.dma_start(out=b1t[:], in_=b1td[:])
            b1p1t = const.tile([128, H1 * C1 // 128], F32)
            nc.sync.dma_start(out=b1p1t[:], in_=b1p1td[:])
            b2ct = const.tile([128, 4], F32)
            nc.sync.dma_start(out=b2ct[:], in_=b2ctd[:])
            b2cp1t = const.tile([128, 4], F32)
            nc.sync.dma_start(out=b2cp1t[:], in_=b2cp1td[:])
            csdt = const.tile([128, 1], F32)
            nc.sync.dma_start(out=csdt[:], in_=csdtd[:])
            a2sb = const.tile([128, 512], F32)
            nc.sync.dma_start(out=a2sb[:], in_=a2sbd[:])
            a2db = const.tile([128, 512], F32)
            nc.sync.dma_start(out=a2db[:], in_=a2dbd[:])
            ones = const.tile([128, 1], F16)
            nc.vector.memset(ones[:], 1.0)
            w1t = const.tile([D0, H1 * C1], F16)
            nc.sync.dma_start(out=w1t[:], in_=W1f[:])
            # head weight slabs [128, MD] per K-chunk + bias tiles
            hws = {}
            hbt = {}
            for nm, KD, MD in HEAD_LAYERS:
                if nm not in ("r1", "r2"):
                    kc = KD // 128
                    hws[nm] = []
                    for i in range(kc):
                        t = headw.tile([128, MD], F16, tag=f"hw_{nm}_{i}", name=f"hw_{nm}_{i}")
                        nc.sync.dma_start(out=t[:],
                                          in_=Wt[nm][i * 128:(i + 1) * 128, :])
                        hws[nm].append(t)
                mw = min(128, MD)
                mc = (MD + 127) // 128
                bt = headw.tile([mw, mc], F32, tag=f"hb_{nm}", name=f"hb_{nm}")
                nc.sync.dma_start(
                    out=bt[:], in_=Bt[nm].rearrange("(a p) o -> p (a o)", p=mw))
                hbt[nm] = bt
            cT_t = []
            for i in range(8):
                t = const.tile([128, GPC], F16, tag=f"cT{i}", name=f"cT{i}")
                nc.sync.dma_start(out=t[:], in_=cellT[i * 128:(i + 1) * 128, :])
                cT_t.append(t)

            aggT = [aggp.tile([D0, 768], F16, tag=f"aggT{h}", name=f"aggT{h}") for h in range(H1)]
            o2T = [o2p.tile([128, 768], F16, tag=f"o2T{cc}", name=f"o2T{cc}") for cc in range(4)]
            vT = {k: o2p.tile([128, GPC], F16, tag=f"vT{k}", name=f"vT{k}") for k in (1, 2)}
            # csd gate: rewritten at the end of fused_l2(2) so l2agg's logit
            # chain cannot be scheduled ahead of branch-2's fused phase
            csd_gate = const.tile([128, 1], F32, tag="csd_gate", name="csd_gate")
            adcols = {k: const.tile([128, NBLK], F32, tag=f"adcols{k}", name=f"adcols{k}")
                      for k in (1, 2)}

            # ---------------- helpers
            def elu1(ps_ap, bias_col, biasp1_col, out_ap, n, alt=False):
                """out = ELU(ps + bias) + 1 = min(exp(y), relu(y)+1), f16.
                alt=True shifts the relu-part from Act to DVE for balance."""
                ex = sb.tile([128, n], F16, tag=f"elu_ex{n}", name=f"elu_ex{n}")
                nc.scalar.activation(ex[:], ps_ap, Act.Exp, bias=bias_col)
                r0 = sb.tile([128, n], F16, tag=f"elu_r0{n}", name=f"elu_r0{n}")
                if alt:
                    nc.vector.tensor_scalar(out=r0[:], in0=ps_ap,
                                            scalar1=biasp1_col, scalar2=1.0,
                                            op0=Alu.add, op1=Alu.max)
                    nc.vector.tensor_tensor(out=out_ap, in0=r0[:], in1=ex[:],
                                            op=Alu.min)
                else:
                    nc.scalar.activation(r0[:], ps_ap, Act.Relu, bias=bias_col)
                    nc.vector.scalar_tensor_tensor(
                        out=out_ap, in0=r0[:], scalar=1.0, in1=ex[:],
                        op0=Alu.add, op1=Alu.min)

            def dense_stream(xtiles, nm, md):
                """k-outer / m-inner dense with streamed weight slabs."""
                kc = len(xtiles)
                mc = (md + 127) // 128
                outs = []
                bt = hbt[nm]
                for g0 in range(0, mc, 8):
                    gmc = min(8, mc - g0)
                    with tc.tile_pool(name=f"psd{nm}{g0}", bufs=1,
                                      space="PSUM") as psd:
                        pds = [psd.tile([128, GPC], F32, tag=f"pd{nm}{m}",
                                        name=f"pd{nm}{m}") for m in range(gmc)]
                        for i in range(kc):
                            ws = stream.tile([128, 1024], F16, tag=f"ws{nm}",
                                             name=f"ws{nm}")
                            nc.sync.dma_start(
                                out=ws[:, 0:gmc * 128],
                                in_=Wt[nm][i * 128:(i + 1) * 128,
                                           g0 * 128:(g0 + gmc) * 128])
                            for m in range(gmc):
                                nc.tensor.matmul(
                                    pds[m][:], ws[:, m * 128:(m + 1) * 128],
                                    xtiles[i][:], start=(i == 0),
                                    stop=(i == kc - 1))
                        for m in range(gmc):
                            o = o2p.tile([128, GPC], F16, tag=f"do{nm}{g0+m}",
                                         name=f"do{nm}{g0+m}")
                            nc.scalar.activation(o[:], pds[m][:], Act.Relu,
                                                 bias=bt[:, g0 + m:g0 + m + 1])
                            outs.append(o)
                return outs

            def dense(xtiles, nm, md, act=True, out_f32=False):
                kc = len(xtiles)
                mc = (md + 127) // 128
                outs = []
                bt = hbt[nm]
                with tc.tile_pool(name=f"psd{nm}", bufs=2, space="PSUM") as psd:
                    for m in range(mc):
                        mw = min(128, md - m * 128)
                        pd = psd.tile([mw, GPC], F32, tag=f"pd{nm}", name=f"pd{nm}")
                        for i in range(kc):
                            nc.tensor.matmul(
                                pd[:], hws[nm][i][:, m * 128:m * 128 + mw],
                                xtiles[i][:], start=(i == 0), stop=(i == kc - 1))
                        o = o2p.tile([mw, GPC], F32 if out_f32 else F16,
                                     tag=f"do{nm}{m}", name=f"do{nm}{m}")
                        if act:
                            nc.scalar.activation(o[:], pd[:], Act.Relu,
                                                 bias=bt[0:mw, m:m + 1])
                        else:
                            nc.vector.tensor_scalar(out=o[:], in0=pd[:],
                                                    scalar1=bt[0:mw, m:m + 1],
                                                    scalar2=None, op0=Alu.add)
                        outs.append(o)
                return outs

            def l1agg(k):
                """L1 edge-softmax aggregation in x-space -> aggT (f16)."""
                with (
                    tc.tile_pool(name=f"psagg{k}", bufs=2, space="PSUM") as psagg,
                    tc.tile_pool(name=f"pstr{k}", bufs=2, space="PSUM") as pstr,
                ):
                    ch0 = 0
                    for b in range(NBLK):
                        nchb = n_ch[k][b]
                        xgb = xgpool.tile([128, NCHB * XTW], F16, tag="xgb")
                        for ci in range(nchb):
                            if skip_gather:
                                nc.gpsimd.dma_start(
                                    out=xgb[:, ci * XTW:(ci + 1) * XTW],
                                    in_=xtab[k][(ci % 40) * 128:(ci % 40) * 128 + 128, :])
                            else:
                                nc.gpsimd.indirect_dma_start(
                                    out=xgb[:, ci * XTW:(ci + 1) * XTW],
                                    out_offset=None, in_=xtab[k],
                                    in_offset=IndirectOffsetOnAxis(
                                        ap=sit[k][:, ch0 + ci:ch0 + ci + 1], axis=0))
                        xv = xgb[:].rearrange("p (c f) -> p c f", f=XTW)
                        # attention weights for the whole block
                        e0b = sb.tile([128, NCHB * H1], F16, tag="e0b")
                        nc.vector.tensor_tensor(
                            out=e0b[:, 0:nchb * H1].rearrange(
                                "p (c h) -> p c h", h=H1),
                            in0=xv[:, 0:nchb, 79:89],
                            in1=adpe[k][:, ch0 * H1:(ch0 + nchb) * H1].rearrange(
                                "p (c h) -> p c h", h=H1),
                            op=Alu.add)
                        t1b = sb.tile([128, NCHB * H1], F16, tag="t1b")
                        nc.vector.tensor_scalar_mul(
                            t1b[:, 0:nchb * H1], e0b[:, 0:nchb * H1], NEG)
                        t2b = sb.tile([128, NCHB * H1], F16, tag="t2b")
                        nc.vector.tensor_tensor(
                            out=t2b[:, 0:nchb * H1], in0=t1b[:, 0:nchb * H1],
                            in1=e0b[:, 0:nchb * H1], op=Alu.max)
                        wb = sb.tile([128, NCHB * H1], F16, tag="wb")
                        nc.scalar.activation(wb[:, 0:nchb * H1],
                                             t2b[:, 0:nchb * H1], Act.Exp)
                        wv = wb[:].rearrange("p (c h) -> p c h", h=H1)
                        ps = psagg.tile([128, H1 * 79], F32, tag="agg")
                        for ci in range(nchb):
                            ch = ch0 + ci
                            oh = sb.tile([128, 128], F16, tag="oh")
                            nc.vector.tensor_scalar(
                                out=oh[:], in0=iota[:],
                                scalar1=dlt[k][:, ch:ch + 1], scalar2=None,
                                op0=Alu.is_equal)
                            wxg = sb.tile([128, H1, 79], F16, tag="wxg")
                            nc.vector.tensor_tensor(
                                out=wxg[:],
                                in0=xv[:, ci:ci + 1, 0:79].broadcast_to(
                                    [128, H1, 79]),
                                in1=wv[:, ci, :].rearrange(
                                    "p (h o) -> p h o", o=1).broadcast_to(
                                        [128, H1, 79]),
                                op=Alu.mult)
                            wxg2 = wxg[:].rearrange("p h f -> p (h f)")
                            nc.tensor.matmul(ps[:, 0:512], oh[:],
                                             wxg2[:, 0:512],
                                             start=(ci == 0),
                                             stop=(ci == nchb - 1))
                            nc.tensor.matmul(ps[:, 512:790], oh[:],
                                             wxg2[:, 512:790],
                                             start=(ci == 0),
                                             stop=(ci == nchb - 1))
                        den = sb.tile([128, H1], F32, tag="den")
                        nc.vector.tensor_scalar_max(
                            den[:],
                            ps[:].rearrange("p (h f) -> p h f", f=79)[:, :, 78],
                            1e-30)
                        rec = sb.tile([128, H1], F32, tag="rec")
                        nc.vector.reciprocal(out=rec[:], in_=den[:])
                        for h in range(H1):
                            sc = sb.tile([128, D0], F16, tag="sc")
                            nc.vector.tensor_scalar(
                                out=sc[:], in0=ps[:, h * 79:h * 79 + D0],
                                scalar1=rec[:, h:h + 1], scalar2=None,
                                op0=Alu.mult)
                            tp = pstr.tile([D0, 128], F16, tag="tp")
                            nc.tensor.transpose(out=tp[:], in_=sc[:],
                                                identity=ident[:])
                            nc.scalar.copy(
                                out=aggT[h][:, b * 128:(b + 1) * 128],
                                in_=tp[:])
                        ch0 += nchb

            def fused_l2(k):
                """h1+1 = ELU(aggT@W1+b1)+1 per 128-feature chunk, immediately
                consumed by the W2 (and attention-logit) accumulation."""
                with (
                    tc.tile_pool(name=f"psmm{k}", bufs=1, space="PSUM") as psmm,
                    tc.tile_pool(name=f"psfin{k}", bufs=2, space="PSUM") as psfin,
                ):
                    pm = [psmm.tile([128, 512], F32, tag=f"pm{m}", name=f"pm{m}")
                          for m in range(6)]

                    def consume(i, h1t, w2w, t):
                        woff = t * 512
                        for m in range(6):
                            lhs = h1t[:, m * 128:(m + 1) * 128]
                            nc.tensor.matmul(pm[m][:], lhs,
                                             w2w[:, woff:woff + 512],
                                             start=(i == 0), stop=(i == 79))

                    prev = None
                    for j in range(40):
                        w2w = stream.tile([128, 1024], F16, tag="w2w")
                        nc.sync.dma_start(
                            out=w2w[:].rearrange("p (a o) -> p a o", a=2),
                            in_=W2aug[j * 256:(j + 1) * 256, :].rearrange(
                                "(a p) o -> p a o", p=128))
                        for t in range(2):
                            i = 2 * j + t
                            h = i // 8
                            h1t = stream.tile([128, 768], F16, tag="h1t")
                            for half in range(2):
                                pf = psfin.tile([128, 384], F32, tag="pf")
                                nc.tensor.matmul(
                                    pf[:], w1t[:, i * 128:(i + 1) * 128],
                                    aggT[h][:, half * 384:(half + 1) * 384],
                                    start=True, stop=True)
                                elu1(pf[:], b1t[:, i:i + 1], b1p1t[:, i:i + 1],
                                     h1t[:, half * 384:(half + 1) * 384], 384,
                                     alt=(half == 0))
                            # software pipeline: PE consumes chunk i-1 while
                            # chunk i's ELU is in flight
                            if prev is not None:
                                consume(*prev)
                            prev = (i, h1t, w2w, t)
                    consume(*prev)
                    # pack the node table + logits (weighted row-sums of h2)
                    for m in range(6):
                        nrow = 128 if m < 5 else NV - 640
                        tmp = sb.tile([128, 512], F32, tag="lgt")
                        nc.vector.tensor_tensor(out=tmp[:], in0=pm[m][:],
                                                in1=a2sb[:], op=Alu.mult)
                        asr = sb.tile([128, 1], F32, tag="asr")
                        nc.vector.tensor_reduce(out=asr[:], in_=tmp[:],
                                                axis=mybir.AxisListType.X,
                                                op=Alu.add)
                        nc.vector.tensor_tensor(out=tmp[:], in0=pm[m][:],
                                                in1=a2db[:], op=Alu.mult)
                        adr = sb.tile([128, 1], F32, tag="adr")
                        nc.vector.tensor_reduce(out=adr[:], in_=tmp[:],
                                                axis=mybir.AxisListType.X,
                                                op=Alu.add)
                        nc.vector.tensor_copy(out=adcols[k][:, m:m + 1],
                                              in_=adr[:])
                        loc = sb.tile([128, T2W], F16, tag="loc")
                        nc.scalar.copy(out=loc[:, 0:512], in_=pm[m][:])
                        nc.vector.memset(loc[:, 512:513], 1.0)
                        nc.vector.memset(loc[:, 513:514], 0.0)
                        nc.vector.tensor_copy(
                            out=loc[:, 514:516].bitcast(F32), in_=asr[:])
                        nc.vector.memset(loc[:, 516:T2W], 0.0)
                        # SWDGE write: keeps the AllGather's wait on DMASW
                        # lanes (gathers only), not the busy HWDGE stream lanes
                        nc.gpsimd.dma_start(
                            out=loc2[k][m * 128:m * 128 + nrow, :],
                            in_=loc[0:nrow, :])
                    nc.gpsimd.dma_start(
                        out=adstvd[k].rearrange("(a p) o -> p (a o)", p=128),
                        in_=adcols[k][:])
                    if k == 2:
                        # csd_gate = csd + 0*adcols2: numerically just csd, but
                        # the read of adcols[2] is a REAL dependency on branch-2
                        # fused output, so the scheduler cannot hoist l2agg's
                        # logit/matmul chain ahead of fused_l2(2)
                        nc.vector.scalar_tensor_tensor(
                            out=csd_gate[:], in0=adcols[2][:, 5:6],
                            scalar=0.0, in1=csdt[:],
                            op0=Alu.mult, op1=Alu.add)

            def l2gather(k):
                tiles = []
                adgs = []
                ch0 = 0
                for b in range(NBLK):
                    nchb = n_ch[k][b]
                    nbmax = max(n_ch[1][b], n_ch[2][b])
                    # adst values for this block's edges (prefetched before
                    # the other branch's AllGather occupies the queue)
                    adgb = hgpool.tile([128, NCHB], F32, tag=f"adgb{b}",
                                       name=f"adgb{b}")
                    for ci in range(nchb):
                        nc.gpsimd.indirect_dma_start(
                            out=adgb[:, ci:ci + 1], out_offset=None,
                            in_=adstvd[k],
                            in_offset=IndirectOffsetOnAxis(
                                ap=dit[k][:, ch0 + ci:ch0 + ci + 1], axis=0))
                    adgs.append(adgb)
                    hgb = hgpool.tile([128, nbmax * T2W], F16,
                                      tag=f"hgb{b}", name=f"hgb{b}")
                    for ci in range(nchb):
                        if skip_gather:
                            nc.gpsimd.dma_start(
                                out=hgb[:, ci * T2W:(ci + 1) * T2W],
                                in_=shared2[k][(ci % 11) * 128:(ci % 11) * 128 + 128, :])
                        else:
                            nc.gpsimd.indirect_dma_start(
                                out=hgb[:, ci * T2W:(ci + 1) * T2W],
                                out_offset=None, in_=shared2[k],
                                in_offset=IndirectOffsetOnAxis(
                                    ap=sit[k][:, ch0 + ci:ch0 + ci + 1], axis=0))
                    tiles.append(hgb)
                    ch0 += nchb
                return tiles, adgs

            def l2agg(k, hgtiles, adgtiles):
                with (
                    tc.tile_pool(name=f"psag2{k}", bufs=1, space="PSUM") as psag2,
                    tc.tile_pool(name=f"pstr2{k}", bufs=2, space="PSUM") as pstr2,
                ):
                    ch0 = 0
                    for b in range(NBLK):
                        nchb = n_ch[k][b]
                        hgb = hgtiles[b]
                        hv = hgb[:].rearrange("p (c f) -> p c f", f=T2W)
                        ps = psag2.tile([128, 513], F32, tag="agg2")
                        # edge logits for the whole block in one shot:
                        # e = asrc[src] + csd + adst[dst]; adst prefetched
                        # from the DRAM adstv table by local dst index.
                        adgb = adgtiles[b]
                        e0b2 = sb.tile([128, NCHB], F32, tag="e0b2")
                        for ci in range(nchb):
                            nc.vector.scalar_tensor_tensor(
                                out=e0b2[:, ci:ci + 1],
                                in0=hv[:, ci, 514:516].bitcast(F32),
                                scalar=csd_gate[:, 0:1],
                                in1=adgb[:, ci:ci + 1],
                                op0=Alu.add, op1=Alu.add)
                        t1c = sb.tile([128, NCHB], F32, tag="t1c")
                        nc.vector.tensor_scalar_mul(
                            t1c[:, 0:nchb], e0b2[:, 0:nchb], NEG)
                        t2c = sb.tile([128, NCHB], F32, tag="t2c")
                        nc.vector.tensor_tensor(
                            out=t2c[:, 0:nchb], in0=t1c[:, 0:nchb],
                            in1=e0b2[:, 0:nchb], op=Alu.max)
                        w2b = sb.tile([128, NCHB], F32, tag="w2b")
                        nc.scalar.activation(w2b[:, 0:nchb], t2c[:, 0:nchb],
                                             Act.Exp)
                        for ci in range(nchb):
                            ch = ch0 + ci
                            oh = sb.tile([128, 128], F16, tag="oh")
                            nc.vector.tensor_scalar(
                                out=oh[:], in0=iota[:],
                                scalar1=dlt[k][:, ch:ch + 1], scalar2=None,
                                op0=Alu.is_equal)
                            wh = sb.tile([128, 513], F16, tag="wh")
                            nc.vector.tensor_scalar(
                                out=wh[:], in0=hv[:, ci, 0:513],
                                scalar1=w2b[:, ci:ci + 1], scalar2=None,
                                op0=Alu.mult)
                            nc.tensor.matmul(ps[:, 0:512], oh[:],
                                             wh[:, 0:512],
                                             start=(ci == 0),
                                             stop=(ci == nchb - 1))
                            nc.tensor.matmul(ps[:, 512:513], oh[:],
                                             wh[:, 512:513],
                                             start=(ci == 0),
                                             stop=(ci == nchb - 1))
                        den = sb.tile([128, 1], F32, tag="dn2")
                        nc.vector.tensor_scalar_max(den[:], ps[:, 512:513],
                                                    1e-30)
                        rec = sb.tile([128, 1], F32, tag="rc2")
                        nc.vector.reciprocal(out=rec[:], in_=den[:])
                        for cc in range(4):
                            sc = sb.tile([128, 128], F16, tag="sc2")
                            nc.vector.tensor_scalar(
                                out=sc[:], in0=ps[:, cc * 128:(cc + 1) * 128],
                                scalar1=rec[:, 0:1], scalar2=None, op0=Alu.mult)
                            tp = pstr2.tile([128, 128], F16, tag="tp2")
                            nc.tensor.transpose(out=tp[:], in_=sc[:],
                                                identity=ident[:])
                            elu1(tp[:], b2ct[:, cc:cc + 1], b2cp1t[:, cc:cc + 1],
                                 o2T[cc][:, b * 128:(b + 1) * 128], 128,
                                 alt=(cc % 4 == 3))
                        ch0 += nchb

            def pool_wg(k):
                with tc.tile_pool(name=f"psg{k}", bufs=2, space="PSUM") as psg:
                    gT = sb.tile([128, 4, GPC], F16, tag="gT")
                    for cc in range(4):
                        for g in range(GPC):
                            nc.vector.tensor_reduce(
                                out=gT[:, cc, g:g + 1],
                                in_=o2T[cc][:, bounds[g]:bounds[g + 1]],
                                axis=mybir.AxisListType.X, op=Alu.max)
                    pg = psg.tile([128, GPC], F32, tag="pg")
                    for kc in range(4):
                        nc.tensor.matmul(pg[:], hws["g"][kc][:], gT[:, kc, :],
                                         start=(kc == 0), stop=(kc == 3))
                    nc.scalar.activation(vT[k][:], pg[:], Act.Relu,
                                         bias=hbt["g"][:, 0:1])

            def l2norm_scale(xtiles, tag):
                n = len(xtiles)
                with tc.tile_pool(name=f"psn{tag}", bufs=1, space="PSUM") as psn:
                    pn = psn.tile([1, GPC], F32, tag=f"pn{tag}", name=f"pn{tag}")
                    for i in range(n):
                        sq = sb.tile([128, GPC], F16, tag="sq")
                        nc.scalar.activation(sq[:], xtiles[i][:], Act.Square)
                        nc.tensor.matmul(pn[:], ones[:], sq[:],
                                         start=(i == 0), stop=(i == n - 1))
                    nrm = sb.tile([1, GPC], F32, tag=f"nr{tag}", name=f"nr{tag}")
                    nc.scalar.activation(nrm[:], pn[:], Act.Sqrt)
                    nc.vector.tensor_scalar_max(nrm[:], nrm[:], 1e-12)
                    rcp = sb.tile([1, GPC], F32, tag=f"rcn{tag}", name=f"rcn{tag}")
                    nc.vector.reciprocal(out=rcp[:], in_=nrm[:])
                    rb = sb.tile([128, GPC], F32, tag=f"rb{tag}", name=f"rb{tag}")
                    nc.gpsimd.partition_broadcast(rb[:], rcp[:])
                    outs = []
                    for i in range(n):
                        o = o2p.tile([128, GPC], F16, tag=f"no{tag}{i}", name=f"no{tag}{i}")
                        nc.vector.tensor_tensor(out=o[:], in0=xtiles[i][:],
                                                in1=rb[:], op=Alu.mult)
                        outs.append(o)
                    return outs

            # ---------------- body
            def _body():
                # emit in READINESS order: the tile scheduler builds engine
                # queues greedily, so program order ~ readiness order avoids
                # both sem-lane pollution and head-of-line blocking.
                l1agg(1)
                fused_l2(1)
                # AG1 immediately after its input is ready; overlaps
                # l1agg(2)+fused_l2(2)
                if skip_cc:
                    nc.sync.dma_start(out=shared2[1][0:NV, :], in_=loc2[1][:])
                else:
                    with tc.high_priority():
                        nc.gpsimd.collective_compute(
                            "AllGather", Alu.bypass,
                            replica_groups=[list(range(N_CORES))],
                            ins=[loc2[1][:].opt()], outs=[shared2[1][:].opt()])
                l1agg(2)
                # cell MLP (independent; PE work fills the gap here)
                r1 = dense_stream(cT_t, "r1", 2048)
                r2 = dense_stream(r1, "r2", 512)
                r3 = dense(r2, "r3", 256)
                # branch-1 L2 gathers depend only on AG1; their Pool-queue
                # slot is before AG2 so they run during fused_l2(2)
                hg1, ad1 = l2gather(1)
                fused_l2(2)
                if skip_cc:
                    nc.sync.dma_start(out=shared2[2][0:NV, :], in_=loc2[2][:])
                else:
                    with tc.high_priority():
                        nc.gpsimd.collective_compute(
                            "AllGather", Alu.bypass,
                            replica_groups=[list(range(N_CORES))],
                            ins=[loc2[2][:].opt()], outs=[shared2[2][:].opt()])
                hg2, ad2 = l2gather(2)
                l2agg(1, hg1, ad1)
                pool_wg(1)
                l2agg(2, hg2, ad2)
                pool_wg(2)
                # head
                xn = l2norm_scale([vT[1], vT[2], r3[0], r3[1]], "x")
                f1 = dense(xn, "f1", 1024)
                f2 = dense(f1, "f2", 512)
                f3 = dense(f2, "f3", 128)
                fo = dense(f3, "o", 2, act=False, out_f32=True)
                nc.sync.dma_start(out=outT[:], in_=fo[0][:])

            for _rep in range(repeat):
                _body()

    nc.compile()
    return nc


def make_in_maps(Hh):
    ins = []
    for c in range(N_CORES):
        m = {}
        for k, v in Hh.items():
            if k in ("srcidx1", "dstloc1", "dstidx1", "adpe1",
                     "srcidx2", "dstloc2", "dstidx2", "adpe2", "cellT"):
                m[k] = np.ascontiguousarray(v[c])
            else:
                m[k] = v
        ins.append(m)
    return ins


# ---------------------------------------------------------------------- runner
import time
import jax
from jax.sharding import Mesh, PartitionSpec
from jax.experimental.shard_map import shard_map

from concourse import bass2jax
from concourse.bass2jax import _bass_exec_p, install_neuronx_cc_hook


class SpmdRunner:
    def __init__(self, nc, n_cores: int):
        install_neuronx_cc_hook()
        self.nc = nc
        self.n_cores = n_cores
        partition_name = nc.partition_id_tensor.name if nc.partition_id_tensor else None
        in_names, out_names, out_avals, zero_outs = [], [], [], []
        for alloc in nc.m.functions[0].allocations:
            if not isinstance(alloc, mybir.MemoryLocationSet):
                continue
            name = alloc.memorylocations[0].name
            if alloc.kind == "ExternalInput":
                if name != partition_name:
                    in_names.append(name)
            elif alloc.kind == "ExternalOutput":
                out_names.append(name)
                shape = tuple(alloc.tensor_shape)
                dtype = mybir.dt.np(alloc.dtype)
                out_avals.append(jax.core.ShapedArray(shape, dtype))
                zero_outs.append(np.zeros(shape, dtype))
        self.in_names = list(in_names)
        self.out_names = out_names
        self.out_avals = out_avals
        self.zero_outs = zero_outs
        n_params = len(in_names)
        self.n_params = n_params
        all_in_names = list(in_names) + list(out_names)
        if partition_name is not None:
            all_in_names.append(partition_name)

        def _body(*args):
            operands = list(args)
            if partition_name is not None:
                operands.append(bass2jax.partition_id_tensor())
            outs = _bass_exec_p.bind(
                *operands,
                out_avals=tuple(out_avals),
                in_names=tuple(all_in_names),
                out_names=tuple(out_names),
                lowering_input_output_aliases=(),
                sim_require_finite=True,
                sim_require_nnan=True,
                nc=nc,
            )
            return tuple(outs)

        donate = tuple(range(n_params, n_params + len(out_names)))
        devices = jax.devices()[:n_cores]
        mesh = Mesh(np.asarray(devices), ("core",))
        in_specs = (PartitionSpec("core"),) * (n_params + len(out_names))
        out_specs = (PartitionSpec("core"),) * len(out_names)
        self._fn = jax.jit(
            shard_map(_body, mesh=mesh, in_specs=in_specs, out_specs=out_specs,
                      check_rep=False),
            donate_argnums=donate, keep_unused=True)

    def _concat_inputs(self, in_maps):
        per_core = [[np.asarray(m[n]) for n in self.in_names] for m in in_maps]
        return [np.concatenate([per_core[c][i] for c in range(self.n_cores)], axis=0)
                for i in range(self.n_params)]

    def _zeros(self):
        return [np.zeros((self.n_cores * z.shape[0], *z.shape[1:]), z.dtype)
                for z in self.zero_outs]

    def run(self, in_maps):
        concat_in = self._concat_inputs(in_maps)
        outs = self._fn(*concat_in, *self._zeros())
        res = []
        for c in range(self.n_cores):
            d = {}
            for i, name in enumerate(self.out_names):
                d[name] = np.asarray(outs[i]).reshape(
                    self.n_cores, *self.out_avals[i].shape)[c]
            res.append(d)
        return res

    def time(self, in_maps, iters=20, warmup=3, inner=5):
        """Returns (best_per_call_s, all_times). Dispatches `inner` calls
        back-to-back then blocks, to amortize host->terminal latency."""
        concat_in = [jax.device_put(x) for x in self._concat_inputs(in_maps)]
        times = []
        for it in range(warmup + iters):
            zs = [self._zeros() for _ in range(inner)]
            t0 = time.perf_counter()
            outs = None
            for k in range(inner):
                outs = self._fn(*concat_in, *zs[k])
            jax.block_until_ready(outs)
            dt = (time.perf_counter() - t0) / inner
            if it >= warmup:
                times.append(dt)
        return min(times), times


# ---------------------------------------------------------------- entry point
_CACHE = {}


def _get_runner(Hh, meta):
    key = (tuple(meta["n_ch1"]), tuple(meta["n_ch2"]), tuple(meta["bounds1"]))
    ent = _CACHE.get(key)
    if ent is None:
        nc = build(Hh, meta)
        ent = SpmdRunner(nc, N_CORES)
        _CACHE[key] = ent
    return ent


def kernel(**inputs):
    Hh, meta = host_prep(inputs)
    runner = _get_runner(Hh, meta)
    res = runner.run(make_in_maps(Hh))
    out = np.concatenate([res[c]["outT"].T for c in range(N_CORES)], axis=0)
    return out.astype(np.float32)

